# revision 35
# baseline (speedup 1.0000x reference)
"""Sparse attention (masked MHA) distributed over 8 TRN2 NeuronCores.

Sharding: (batch=4) x (head-half=2) -> 8 cores. Core c handles batch
c//2 and heads [8*(c%2), 8*(c%2)+8). Host slices the weights per core
(tensor parallelism); x_q/x_kv/mask ship full per batch.

Per core: project Q/K/V for its 8 heads over the full 2048-token
sequence, masked softmax attention (scores transposed, no max
subtraction, ones-column in V gives the softmax denominator), its
8 heads' slice of z (disjoint output), and a partial output
projection; partners ReduceScatter-add their partial `out` halves
(the only collective).

Layouts mirror the q-sharded v1 (kernel_v1_seqshard.py): x^T via
batched PE transposes, Q^T/K^T in head-pair rows [128=2x64, seq],
S^T[k,q] with row-tiled K=64 pair matmuls, exp over [128,1024]
2-bank PSUM tiles, post-exp 0/1 mask multiply.
"""

import numpy as np
import ml_dtypes

import concourse.bass as bass
import concourse.mybir as mybir
import concourse.tile as tile
from concourse import bacc
from concourse.masks import make_identity

F32 = mybir.dt.float32
BF16 = mybir.dt.bfloat16

B, S, DM, NH, DH = 4, 2048, 1024, 16, 64
NHO = NH // 2        # heads per core = 8
NP = NHO // 2        # head pairs per core = 4
DC = DM // 128       # d_model chunks = 8
KC = S // 128        # kv chunks = 16
SQ = S // 2          # reduce-scatter half = 1024
HD_OWN = NHO * DH    # own flattened head dim = 512
SCALE = 1.0 / np.sqrt(DH)
GROUPS = [[0, 1], [2, 3], [4, 5], [6, 7]]

_cache = {}
last_exec_time_ns = None
last_results = None


def _build(has_bias: bool, phases: int = 3, use_cc: bool = True):
    nc = bacc.Bacc(
        "TRN2", target_bir_lowering=False, debug=False, num_devices=8
    )

    x_q = nc.dram_tensor("x_q", [S, DM], F32, kind="ExternalInput").ap()
    x_kv = nc.dram_tensor("x_kv", [S, DM], F32, kind="ExternalInput").ap()
    maskT = nc.dram_tensor("maskT", [S, S], BF16, kind="ExternalInput").ap()
    W_Q = nc.dram_tensor("W_Q", [NHO, DM, DH], F32, kind="ExternalInput").ap()
    W_K = nc.dram_tensor("W_K", [NHO, DM, DH], F32, kind="ExternalInput").ap()
    W_V = nc.dram_tensor("W_V", [NHO, DM, DH], F32, kind="ExternalInput").ap()
    W_O = nc.dram_tensor("W_O", [NHO, DH, DM], F32, kind="ExternalInput").ap()
    if has_bias:
        b_Q = nc.dram_tensor("b_Q", [NHO, DH], F32, kind="ExternalInput").ap()
        b_K = nc.dram_tensor("b_K", [NHO, DH], F32, kind="ExternalInput").ap()
        b_V = nc.dram_tensor("b_V", [NHO, DH], F32, kind="ExternalInput").ap()
        b_O = nc.dram_tensor("b_O", [DM], F32, kind="ExternalInput").ap()
    out_d = nc.dram_tensor("out", [SQ, DM], F32, kind="ExternalOutput").ap()
    z_d = nc.dram_tensor("z", [S, HD_OWN], F32, kind="ExternalOutput").ap()

    WOr = W_O.rearrange("n h d -> (n h) d")  # [512, 1024]

    with tile.TileContext(nc) as tc:
        with (
            tc.tile_pool(name="persist", bufs=1) as persist,
            tc.tile_pool(name="wo_keep", bufs=NP) as wokeep,
            tc.tile_pool(name="rs", bufs=1, space="DRAM") as rs_pool,
            tc.tile_pool(name="ps", bufs=2, space="PSUM") as ps,
            tc.tile_pool(name="ps_pv", bufs=4, space="PSUM") as ps_pv,
        ):
            ident = persist.tile([128, 128], BF16, tag="ident")
            make_identity(nc, ident)

            ones_row = persist.tile([1, 512], BF16, tag="ones_row")
            nc.any.memset(ones_row, 1.0)

            if has_bias:
                bias_sb = {}
                for nm, apv in (("q", b_Q), ("k", b_K), ("v", b_V)):
                    st = persist.tile([1, HD_OWN], F32, tag=f"b_{nm}_f")
                    nc.sync.dma_start(st, apv.rearrange("n h -> (n h)")[None, :])
                    bb = persist.tile([1, HD_OWN], BF16, tag=f"b_{nm}")
                    nc.any.tensor_copy(bb, st)
                    bias_sb[nm] = bb
                st = persist.tile([1, DM], F32, tag="b_o_f")
                nc.sync.dma_start(st, b_O[None, :])
                b_o_half = persist.tile([1, DM], BF16, tag="b_o")
                # each partner adds b_O/2; the ReduceScatter-add restores b_O
                nc.vector.tensor_scalar_mul(b_o_half, st, 0.5)

            # ---------------- Phase 1a: transpose x_q, x_kv (bf16) -------
            with (
                tc.tile_pool(name="xT", bufs=1) as xTp,
                tc.tile_pool(name="stage", bufs=4) as stage,
                tc.tile_pool(name="w_st", bufs=3) as wst,
                tc.tile_pool(name="w_keep", bufs=2 * DC) as wkeep,
            ):
                xTq = xTp.tile([128, DC, S], BF16, tag="xTq", name="xTq")
                xTkv = xTp.tile([128, DC, S], BF16, tag="xTkv", name="xTkv")

                def load_transpose_rc(src_ap, dst, rc):
                    st_f = stage.tile([128, DM], F32, tag="x_f32")
                    nc.sync.dma_start(st_f, src_ap[rc * 128:(rc + 1) * 128, :])
                    st_b = stage.tile([128, DM], BF16, tag="x_bf")
                    nc.any.tensor_copy(st_b, st_f)
                    for half in range(2):
                        pt = ps_pv.tile([128, 512], BF16, tag="ps_pv")
                        for j in range(4):
                            dc = half * 4 + j
                            nc.tensor.transpose(
                                pt[:, j * 128:(j + 1) * 128],
                                st_b[:, dc * 128:(dc + 1) * 128],
                                ident,
                            )
                        nc.any.tensor_copy(
                            dst[:, half * 4:(half + 1) * 4,
                                rc * 128:(rc + 1) * 128],
                            pt.rearrange("p (j c) -> p j c", c=128),
                        )

                def load_w(W, nm):
                    # own-head weights [NHO, 128, 64] chunk -> [128, 512] bf16
                    Wb = []
                    for dc in range(DC):
                        st_f = wst.tile([128, NHO, 64], F32, tag="w_f32")
                        nc.sync.dma_start(
                            st_f,
                            W[:, dc * 128:(dc + 1) * 128, :].rearrange(
                                "n p h -> p n h"
                            ),
                        )
                        wb = wkeep.tile([128, HD_OWN], BF16, tag="w_bf",
                                        name=f"w{nm}{dc}")
                        nc.any.tensor_copy(wb, st_f.rearrange("p n h -> p (n h)"))
                        Wb.append(wb)
                    return Wb

                QTt = [persist.tile([128, S], BF16, tag=f"QT{p}", name=f"QT{p}")
                       for p in range(NP)]
                KTt = [persist.tile([128, S], BF16, tag=f"KT{p}", name=f"KT{p}")
                       for p in range(NP)]
                # V augmented with a ones column per head: [k, h*65 + hd]
                Vg = [persist.tile([128, NHO * (DH + 1)], BF16, tag=f"V{k}",
                                   name=f"V{k}")
                      for k in range(KC)]
                for k in range(KC):
                    nc.any.memset(Vg[k], 1.0)

                def v_proj_tile(k):
                    # out [k_tile 128, hd-own 512] -> scatter to Vg
                    pt = ps.tile([128, 512], F32, tag="ps")
                    for dc in range(DC):
                        nc.tensor.matmul(
                            pt,
                            lhsT=xTkv[:, dc, k * 128:(k + 1) * 128],
                            rhs=WVb[dc],
                            start=(dc == 0),
                            stop=(dc == DC - 1 and not has_bias),
                        )
                    if has_bias:
                        nc.tensor.matmul(
                            pt,
                            lhsT=ones_row[:, :128],
                            rhs=bias_sb["v"],
                            start=False,
                            stop=True,
                        )
                    nc.any.tensor_copy(
                        Vg[k].rearrange("p (h c) -> p h c", c=65)[:, :, 0:64],
                        pt.rearrange("p (h c) -> p h c", c=64),
                    )

                def qk_proj_p(Wb, xT, dst, bias_key, p):
                    # dst[p] [128=pair hd, S] = W_pair^T @ x^T
                    for qt in range(S // 512):
                        pt = ps.tile([128, 512], F32, tag="ps")
                        for dc in range(DC):
                            nc.tensor.matmul(
                                pt,
                                lhsT=Wb[dc][:, p * 128:(p + 1) * 128],
                                rhs=xT[:, dc, qt * 512:(qt + 1) * 512],
                                start=(dc == 0),
                                stop=(dc == DC - 1 and not has_bias),
                            )
                        if has_bias:
                            nc.tensor.matmul(
                                pt,
                                lhsT=bias_sb[bias_key][:, p * 128:(p + 1) * 128],
                                rhs=ones_row[:, :512],
                                start=False,
                                stop=True,
                            )
                        nc.any.tensor_copy(
                            dst[p][:, qt * 512:(qt + 1) * 512], pt
                        )


                # W_V first, then x_kv tiles with V-proj woven per k-tile:
                # each k-tile's projection needs only its own xTkv columns.
                WVb = load_w(W_V, "v")
                for rc in range(KC):
                    load_transpose_rc(x_kv, xTkv, rc)
                    v_proj_tile(rc)
                WKb = load_w(W_K, "k")
                for p in range(NP):
                    qk_proj_p(WKb, xTkv, KTt, "k", p)
                    for rc in range(4 * p, 4 * p + 4):
                        load_transpose_rc(x_q, xTq, rc)
                WQb = load_w(W_Q, "q")

                for p in range(NP):
                    qk_proj_p(WQb, xTq, QTt, "q", p)

            # ---------------- Phase 2+3: attention, z, O-proj, RS ---------
            zT = [persist.tile([128, S], BF16, tag=f"zT{p}", name=f"zT{p}")
                  for p in range(NP)]

            if phases >= 2:
              with (
                tc.tile_pool(name="wo_st", bufs=2) as wost,
                tc.tile_pool(name="mask_p", bufs=KC) as mask_p,
                tc.tile_pool(name="p_sb", bufs=6) as p_sb,
                tc.tile_pool(name="bc", bufs=2) as bc_pool,
                tc.tile_pool(name="rrow", bufs=2) as rrow_pool,
              ):
                maskTt = [mask_p.tile([128, S], BF16, tag="m", name=f"m{k}")
                          for k in range(KC)]
                for k in range(KC):
                    nc.sync.dma_start(maskTt[k], maskT[k * 128:(k + 1) * 128, :])
                WOb = []
                for p in range(NP):
                    st_f = wost.tile([128, DM], F32, tag="wo_f32")
                    nc.sync.dma_start(st_f, WOr[p * 128:(p + 1) * 128, :])
                    wb = wokeep.tile([128, DM], BF16, tag="wo_bf", name=f"wob{p}")
                    nc.any.tensor_copy(wb, st_f)
                    WOb.append(wb)

                rs_in = rs_pool.tile([S, DM], BF16, tag="rs_in")
                rs_out = rs_pool.tile([SQ, DM], BF16, tag="rs_out")

                def phase3_all(o_sb):
                    # z + partial-out, then one bf16 ReduceScatter
                    for qc in range(S // 128):
                        cs = slice(qc * 128, (qc + 1) * 128)
                        z_sb = o_sb.tile([128, HD_OWN], F32, tag="z_sb")
                        for p in range(NP):
                            ptz = ps_pv.tile([128, 128], BF16, tag="ps_pv")
                            nc.tensor.transpose(ptz, zT[p][:, cs], ident)
                            nc.vector.tensor_copy(
                                z_sb[:, p * 128:(p + 1) * 128], ptz
                            )
                        nc.sync.dma_start(z_d[cs, :], z_sb)

                        o_tile = o_sb.tile([128, DM], BF16, tag="o_tile")
                        for dmh in range(2):
                            ds_ = slice(dmh * 512, (dmh + 1) * 512)
                            pt = ps.tile([128, 512], F32, tag="ps")
                            for p in range(NP):
                                nc.tensor.matmul(
                                    pt,
                                    lhsT=zT[p][:, cs],
                                    rhs=WOb[p][:, ds_],
                                    start=(p == 0),
                                    stop=(p == NP - 1 and not has_bias),
                                )
                            if has_bias:
                                nc.tensor.matmul(
                                    pt,
                                    lhsT=ones_row[:, :128],
                                    rhs=b_o_half[:, ds_],
                                    start=False,
                                    stop=True,
                                )
                            nc.any.tensor_copy(o_tile[:, ds_], pt)
                        nc.sync.dma_start(rs_in[cs, :], o_tile)
                    if use_cc:
                        nc.gpsimd.collective_compute(
                            "ReduceScatter",
                            mybir.AluOpType.add,
                            replica_groups=GROUPS,
                            ins=[rs_in.opt()],
                            outs=[rs_out.opt()],
                        )
                    else:
                        nc.sync.dma_start(rs_out, rs_in[:SQ, :])
                    for j in range(SQ // 128):
                        r0 = j * 128
                        fb = o_sb.tile([128, DM], BF16, tag="rs_bf")
                        nc.sync.dma_start(fb, rs_out[r0:r0 + 128, :])
                        ff = o_sb.tile([128, DM], F32, tag="rs_f32")
                        nc.vector.tensor_copy(ff, fb)
                        nc.sync.dma_start(out_d[r0:r0 + 128, :], ff)

                for blk in range(2 * NP):
                    p, qh = blk % NP, blk // NP
                    q0 = qh * SQ
                    pv = [[ps_pv.tile([65, 512], F32, tag="ps_pv",
                                      name=f"ps_pv{blk}_{pr}_{qt}")
                           for qt in range(2)] for pr in range(2)]
                    for k in range(KC):
                        ks = slice(k * 128, (k + 1) * 128)
                        for par in range(2):
                            h = 2 * p + par
                            rs = slice(par * 64, (par + 1) * 64)
                            st = ps.tile([128, SQ], F32, tag="ps")
                            for qt in range(2):
                                nc.tensor.matmul(
                                    st[:, qt * 512:(qt + 1) * 512],
                                    lhsT=KTt[p][rs, ks],
                                    rhs=QTt[p][rs, q0 + qt * 512:
                                               q0 + (qt + 1) * 512],
                                    start=True,
                                    stop=True,
                                    tile_position=(par * 64, 0),
                                )
                            pe = p_sb.tile([128, SQ], BF16, tag="p_exp")
                            nc.scalar.activation(
                                pe, st, mybir.ActivationFunctionType.Exp,
                                scale=SCALE,
                            )
                            pm = p_sb.tile([128, SQ], BF16, tag="p_m")
                            nc.vector.tensor_mul(
                                pm, pe, maskTt[k][:, q0:q0 + SQ]
                            )
                            for qt in range(2):
                                nc.tensor.matmul(
                                    pv[par][qt],
                                    lhsT=Vg[k][:, h * 65:(h + 1) * 65],
                                    rhs=pm[:, qt * 512:(qt + 1) * 512],
                                    start=(k == 0),
                                    stop=(k == KC - 1),
                                )
                    for par in range(2):
                        for qt in range(2):
                            qs = slice(q0 + qt * 512, q0 + (qt + 1) * 512)
                            pvc = p_sb.tile([65, 512], F32, tag="pv_sb")
                            nc.scalar.activation(
                                pvc, pv[par][qt],
                                mybir.ActivationFunctionType.Identity,
                            )
                            rr = rrow_pool.tile([1, 512], F32, tag="rrow")
                            nc.vector.reciprocal(rr, pvc[64:65, :])
                            bc = bc_pool.tile([64, 512], F32, tag="bc")
                            nc.gpsimd.partition_broadcast(bc, rr)
                            nc.vector.tensor_mul(
                                zT[p][par * 64:(par + 1) * 64, qs],
                                pvc[0:64, :],
                                bc,
                            )
              if phases >= 3:
                with tc.tile_pool(name="o_sb", bufs=3) as o_sb:
                    phase3_all(o_sb)

    nc.compile()
    return nc


def _make_runner(nc):
    """Persistent jitted SPMD callable mirroring bass2jax.run_bass_via_pjrt."""
    import jax
    from jax.experimental.shard_map import shard_map
    from jax.sharding import Mesh, PartitionSpec, NamedSharding
    from concourse import bass2jax
    from concourse.bass2jax import _bass_exec_p, partition_id_tensor
    import concourse.mybir as mybir_

    bass2jax.install_neuronx_cc_hook()
    n_cores = 8
    partition_name = (
        nc.partition_id_tensor.name if nc.partition_id_tensor else None
    )
    in_names, out_names, out_avals, zero_outs = [], [], [], []
    for alloc in nc.m.functions[0].allocations:
        if not isinstance(alloc, mybir_.MemoryLocationSet):
            continue
        name = alloc.memorylocations[0].name
        if alloc.kind == "ExternalInput":
            if name != partition_name:
                in_names.append(name)
        elif alloc.kind == "ExternalOutput":
            out_names.append(name)
            shape = tuple(alloc.tensor_shape)
            dtype = mybir_.dt.np(alloc.dtype)
            out_avals.append(jax.core.ShapedArray(shape, dtype))
            zero_outs.append(np.zeros(shape, dtype))
    n_params = len(in_names)
    n_outs = len(out_avals)
    all_in_names = list(in_names) + list(out_names)
    if partition_name is not None:
        all_in_names.append(partition_name)
    donate = tuple(range(n_params, n_params + n_outs))

    def _body(*args):
        operands = list(args)
        if partition_name is not None:
            operands.append(partition_id_tensor())
        outs = _bass_exec_p.bind(
            *operands,
            out_avals=tuple(out_avals),
            in_names=tuple(all_in_names),
            out_names=tuple(out_names),
            lowering_input_output_aliases=(),
            sim_require_finite=True,
            sim_require_nnan=True,
            nc=nc,
        )
        return tuple(outs)

    devices = jax.devices()[:n_cores]
    mesh = Mesh(np.asarray(devices), ("core",))
    sharded = jax.jit(
        shard_map(
            _body, mesh=mesh,
            in_specs=(PartitionSpec("core"),) * (n_params + n_outs),
            out_specs=(PartitionSpec("core"),) * n_outs,
            check_rep=False,
        ),
        donate_argnums=donate,
        keep_unused=True,
    )

    def run(in_maps, bench_iters=0):
        import time as _time
        concat_in = [
            np.concatenate([np.asarray(m[name]) for m in in_maps], axis=0)
            for name in in_names
        ]
        concat_zeros = [
            np.zeros((n_cores * z.shape[0], *z.shape[1:]), z.dtype)
            for z in zero_outs
        ]
        sh = NamedSharding(mesh, PartitionSpec("core"))
        ins_dev = [jax.device_put(a, sh) for a in concat_in]
        out_arrs = sharded(*ins_dev, *concat_zeros)
        jax.block_until_ready(out_arrs)
        results = [
            {
                name: np.asarray(out_arrs[i]).reshape(
                    n_cores, *out_avals[i].shape
                )[c]
                for i, name in enumerate(out_names)
            }
            for c in range(n_cores)
        ]
        times = []
        for _ in range(bench_iters):
            zs = [jax.device_put(np.zeros(
                (n_cores * z.shape[0], *z.shape[1:]), z.dtype), sh)
                for z in zero_outs]
            jax.block_until_ready(zs)
            t0 = _time.perf_counter()
            o = sharded(*ins_dev, *zs)
            jax.block_until_ready(o)
            times.append(_time.perf_counter() - t0)
        return results, (min(times) if times else None)

    return run


def kernel(x_q, x_kv, mask, W_Q, W_K, W_V, W_O, b_Q, b_K, b_V, b_O):
    global last_exec_time_ns, last_results
    import os
    x_q = np.ascontiguousarray(np.asarray(x_q, dtype=np.float32))
    x_kv = np.ascontiguousarray(np.asarray(x_kv, dtype=np.float32))
    mask = np.asarray(mask)
    W_Q, W_K, W_V, W_O = [
        np.ascontiguousarray(np.asarray(w, dtype=np.float32))
        for w in (W_Q, W_K, W_V, W_O)
    ]
    bs = [np.ascontiguousarray(np.asarray(b, dtype=np.float32))
          for b in (b_Q, b_K, b_V, b_O)]
    has_bias = bool(any(np.any(b) for b in bs))

    if has_bias not in _cache:
        _cache[has_bias] = _build(has_bias)
    nc = _cache[has_bias]

    # 0/1 keep-mask transposed to [k, q] (identical on every core)
    keepT = np.ascontiguousarray((~mask).T).astype(ml_dtypes.bfloat16)
    in_maps = []
    for c in range(8):
        b_i, hg = divmod(c, 2)
        hs = slice(hg * NHO, (hg + 1) * NHO)
        m = {
            "x_q": x_q[b_i],
            "x_kv": x_kv[b_i],
            "maskT": keepT,
            "W_Q": np.ascontiguousarray(W_Q[hs]),
            "W_K": np.ascontiguousarray(W_K[hs]),
            "W_V": np.ascontiguousarray(W_V[hs]),
            "W_O": np.ascontiguousarray(W_O[hs]),
        }
        if has_bias:
            m.update({
                "b_Q": np.ascontiguousarray(bs[0][hs]),
                "b_K": np.ascontiguousarray(bs[1][hs]),
                "b_V": np.ascontiguousarray(bs[2][hs]),
                "b_O": bs[3],
            })
        in_maps.append(m)

    key = ("runner", has_bias)
    if key not in _cache:
        _cache[key] = _make_runner(nc)
    bench_iters = int(os.environ.get("BENCH_ITERS", "0"))
    results, tmin = _cache[key](in_maps, bench_iters=bench_iters)
    last_exec_time_ns = None if tmin is None else int(tmin * 1e9)
    last_results = results

    out = np.empty((B, S, DM), np.float32)
    z = np.empty((B, S, NH, DH), np.float32)
    for c in range(8):
        b_i, hg = divmod(c, 2)
        out[b_i, hg * SQ:(hg + 1) * SQ, :] = results[c]["out"]
        z[b_i, :, hg * NHO:(hg + 1) * NHO, :] = (
            results[c]["z"].reshape(S, NHO, DH)
        )
    return out, z


# revision 39
# speedup vs baseline: 1.0060x; 1.0060x over previous
"""Sparse attention (masked MHA) distributed over 8 TRN2 NeuronCores.

Sharding: (batch=4) x (head-half=2) -> 8 cores. Core c handles batch
c//2 and heads [8*(c%2), 8*(c%2)+8). Host slices the weights per core
(tensor parallelism); x_q/x_kv/mask ship full per batch.

Per core: project Q/K/V for its 8 heads over the full 2048-token
sequence, masked softmax attention (scores transposed, no max
subtraction, ones-column in V gives the softmax denominator), its
8 heads' slice of z (disjoint output), and a partial output
projection; partners ReduceScatter-add their partial `out` halves
(the only collective).

Layouts mirror the q-sharded v1 (kernel_v1_seqshard.py): x^T via
batched PE transposes, Q^T/K^T in head-pair rows [128=2x64, seq],
S^T[k,q] with row-tiled K=64 pair matmuls, exp over [128,1024]
2-bank PSUM tiles, post-exp 0/1 mask multiply.
"""

import numpy as np
import ml_dtypes

import concourse.bass as bass
import concourse.mybir as mybir
import concourse.tile as tile
from concourse import bacc
from concourse.masks import make_identity

F32 = mybir.dt.float32
BF16 = mybir.dt.bfloat16

B, S, DM, NH, DH = 4, 2048, 1024, 16, 64
NHO = NH // 2        # heads per core = 8
NP = NHO // 2        # head pairs per core = 4
DC = DM // 128       # d_model chunks = 8
KC = S // 128        # kv chunks = 16
SQ = S // 2          # reduce-scatter half = 1024
HD_OWN = NHO * DH    # own flattened head dim = 512
SCALE = 1.0 / np.sqrt(DH)
GROUPS = [[0, 1], [2, 3], [4, 5], [6, 7]]

_cache = {}
last_exec_time_ns = None
last_results = None


def _build(has_bias: bool, phases: int = 3, use_cc: bool = True):
    nc = bacc.Bacc(
        "TRN2", target_bir_lowering=False, debug=False, num_devices=8
    )

    x_q = nc.dram_tensor("x_q", [S, DM], F32, kind="ExternalInput").ap()
    x_kv = nc.dram_tensor("x_kv", [S, DM], F32, kind="ExternalInput").ap()
    maskT = nc.dram_tensor("maskT", [S, S], BF16, kind="ExternalInput").ap()
    W_Q = nc.dram_tensor("W_Q", [NHO, DM, DH], F32, kind="ExternalInput").ap()
    W_K = nc.dram_tensor("W_K", [NHO, DM, DH], F32, kind="ExternalInput").ap()
    W_V = nc.dram_tensor("W_V", [NHO, DM, DH], F32, kind="ExternalInput").ap()
    W_O = nc.dram_tensor("W_O", [NHO, DH, DM], F32, kind="ExternalInput").ap()
    if has_bias:
        b_Q = nc.dram_tensor("b_Q", [NHO, DH], F32, kind="ExternalInput").ap()
        b_K = nc.dram_tensor("b_K", [NHO, DH], F32, kind="ExternalInput").ap()
        b_V = nc.dram_tensor("b_V", [NHO, DH], F32, kind="ExternalInput").ap()
        b_O = nc.dram_tensor("b_O", [DM], F32, kind="ExternalInput").ap()
    out_d = nc.dram_tensor("out", [SQ, DM], F32, kind="ExternalOutput").ap()
    z_d = nc.dram_tensor("z", [S, HD_OWN], F32, kind="ExternalOutput").ap()

    WOr = W_O.rearrange("n h d -> (n h) d")  # [512, 1024]

    with tile.TileContext(nc) as tc:
        with (
            tc.tile_pool(name="persist", bufs=1) as persist,
            tc.tile_pool(name="wo_keep", bufs=NP) as wokeep,
            tc.tile_pool(name="rs", bufs=1, space="DRAM") as rs_pool,
            tc.tile_pool(name="ps", bufs=2, space="PSUM") as ps,
            tc.tile_pool(name="ps_pv", bufs=4, space="PSUM") as ps_pv,
        ):
            ident = persist.tile([128, 128], BF16, tag="ident")
            make_identity(nc, ident)

            ones_row = persist.tile([1, 512], BF16, tag="ones_row")
            nc.any.memset(ones_row, 1.0)

            if has_bias:
                bias_sb = {}
                for nm, apv in (("q", b_Q), ("k", b_K), ("v", b_V)):
                    st = persist.tile([1, HD_OWN], F32, tag=f"b_{nm}_f")
                    nc.sync.dma_start(st, apv.rearrange("n h -> (n h)")[None, :])
                    bb = persist.tile([1, HD_OWN], BF16, tag=f"b_{nm}")
                    nc.any.tensor_copy(bb, st)
                    bias_sb[nm] = bb
                st = persist.tile([1, DM], F32, tag="b_o_f")
                nc.sync.dma_start(st, b_O[None, :])
                b_o_half = persist.tile([1, DM], BF16, tag="b_o")
                # each partner adds b_O/2; the ReduceScatter-add restores b_O
                nc.vector.tensor_scalar_mul(b_o_half, st, 0.5)

            # ---------------- Phase 1a: transpose x_q, x_kv (bf16) -------
            with (
                tc.tile_pool(name="xT", bufs=1) as xTp,
                tc.tile_pool(name="stage", bufs=4) as stage,
                tc.tile_pool(name="w_st", bufs=3) as wst,
                tc.tile_pool(name="w_keep", bufs=2 * DC) as wkeep,
            ):
                xTq = xTp.tile([128, DC, S], BF16, tag="xTq", name="xTq")
                xTkv = xTp.tile([128, DC, S], BF16, tag="xTkv", name="xTkv")

                def load_transpose_rc(src_ap, dst, rc):
                    st_f = stage.tile([128, DM], F32, tag="x_f32")
                    nc.sync.dma_start(st_f, src_ap[rc * 128:(rc + 1) * 128, :])
                    st_b = stage.tile([128, DM], BF16, tag="x_bf")
                    nc.any.tensor_copy(st_b, st_f)
                    for half in range(2):
                        pt = ps_pv.tile([128, 512], BF16, tag="ps_pv")
                        for j in range(4):
                            dc = half * 4 + j
                            nc.tensor.transpose(
                                pt[:, j * 128:(j + 1) * 128],
                                st_b[:, dc * 128:(dc + 1) * 128],
                                ident,
                            )
                        nc.any.tensor_copy(
                            dst[:, half * 4:(half + 1) * 4,
                                rc * 128:(rc + 1) * 128],
                            pt.rearrange("p (j c) -> p j c", c=128),
                        )

                def load_w(W, nm):
                    # own-head weights [NHO, 128, 64] chunk -> [128, 512] bf16
                    Wb = []
                    for dc in range(DC):
                        st_f = wst.tile([128, NHO, 64], F32, tag="w_f32")
                        nc.sync.dma_start(
                            st_f,
                            W[:, dc * 128:(dc + 1) * 128, :].rearrange(
                                "n p h -> p n h"
                            ),
                        )
                        wb = wkeep.tile([128, HD_OWN], BF16, tag="w_bf",
                                        name=f"w{nm}{dc}")
                        nc.any.tensor_copy(wb, st_f.rearrange("p n h -> p (n h)"))
                        Wb.append(wb)
                    return Wb

                QTt = [persist.tile([128, S], BF16, tag=f"QT{p}", name=f"QT{p}")
                       for p in range(NP)]
                KTt = [persist.tile([128, S], BF16, tag=f"KT{p}", name=f"KT{p}")
                       for p in range(NP)]
                # V augmented with a ones column per head: [k, h*65 + hd]
                Vg = [persist.tile([128, NHO * (DH + 1)], BF16, tag=f"V{k}",
                                   name=f"V{k}")
                      for k in range(KC)]
                for k in range(KC):
                    nc.any.memset(Vg[k], 1.0)

                def v_proj_tile(k):
                    # out [k_tile 128, hd-own 512] -> scatter to Vg
                    pt = ps.tile([128, 512], F32, tag="ps")
                    for dc in range(DC):
                        nc.tensor.matmul(
                            pt,
                            lhsT=xTkv[:, dc, k * 128:(k + 1) * 128],
                            rhs=WVb[dc],
                            start=(dc == 0),
                            stop=(dc == DC - 1 and not has_bias),
                        )
                    if has_bias:
                        nc.tensor.matmul(
                            pt,
                            lhsT=ones_row[:, :128],
                            rhs=bias_sb["v"],
                            start=False,
                            stop=True,
                        )
                    nc.any.tensor_copy(
                        Vg[k].rearrange("p (h c) -> p h c", c=65)[:, :, 0:64],
                        pt.rearrange("p (h c) -> p h c", c=64),
                    )

                def qk_proj_p(Wb, xT, dst, bias_key, p):
                    # dst[p] [128=pair hd, S] = W_pair^T @ x^T
                    for qt in range(S // 512):
                        pt = ps.tile([128, 512], F32, tag="ps")
                        for dc in range(DC):
                            nc.tensor.matmul(
                                pt,
                                lhsT=Wb[dc][:, p * 128:(p + 1) * 128],
                                rhs=xT[:, dc, qt * 512:(qt + 1) * 512],
                                start=(dc == 0),
                                stop=(dc == DC - 1 and not has_bias),
                            )
                        if has_bias:
                            nc.tensor.matmul(
                                pt,
                                lhsT=bias_sb[bias_key][:, p * 128:(p + 1) * 128],
                                rhs=ones_row[:, :512],
                                start=False,
                                stop=True,
                            )
                        nc.any.tensor_copy(
                            dst[p][:, qt * 512:(qt + 1) * 512], pt
                        )


                # First x_kv tiles ahead of W_V so PE transposes start
                # immediately; V-proj woven per k-tile (each k-tile's
                # projection needs only its own xTkv columns).
                for rc in range(3):
                    load_transpose_rc(x_kv, xTkv, rc)
                WVb = load_w(W_V, "v")
                for k in range(3):
                    v_proj_tile(k)
                for rc in range(3, KC):
                    load_transpose_rc(x_kv, xTkv, rc)
                    v_proj_tile(rc)
                WKb = load_w(W_K, "k")
                for p in range(NP):
                    qk_proj_p(WKb, xTkv, KTt, "k", p)
                    for rc in range(4 * p, 4 * p + 4):
                        load_transpose_rc(x_q, xTq, rc)
                WQb = load_w(W_Q, "q")

                for p in range(NP):
                    qk_proj_p(WQb, xTq, QTt, "q", p)

            # ---------------- Phase 2+3: attention, z, O-proj, RS ---------
            zT = [persist.tile([128, S], BF16, tag=f"zT{p}", name=f"zT{p}")
                  for p in range(NP)]

            if phases >= 2:
              with (
                tc.tile_pool(name="wo_st", bufs=2) as wost,
                tc.tile_pool(name="mask_p", bufs=KC) as mask_p,
                tc.tile_pool(name="p_sb", bufs=6) as p_sb,
                tc.tile_pool(name="bc", bufs=2) as bc_pool,
                tc.tile_pool(name="rrow", bufs=2) as rrow_pool,
              ):
                maskTt = [mask_p.tile([128, S], BF16, tag="m", name=f"m{k}")
                          for k in range(KC)]
                for k in range(KC):
                    nc.sync.dma_start(maskTt[k], maskT[k * 128:(k + 1) * 128, :])
                WOb = []
                for p in range(NP):
                    st_f = wost.tile([128, DM], F32, tag="wo_f32")
                    nc.sync.dma_start(st_f, WOr[p * 128:(p + 1) * 128, :])
                    wb = wokeep.tile([128, DM], BF16, tag="wo_bf", name=f"wob{p}")
                    nc.any.tensor_copy(wb, st_f)
                    WOb.append(wb)

                rs_in = rs_pool.tile([S, DM], BF16, tag="rs_in")
                rs_out = rs_pool.tile([SQ, DM], BF16, tag="rs_out")

                def phase3_all(o_sb):
                    # O-proj first -> fire the ReduceScatter, then emit the
                    # (independent) z transposes/stores so they overlap the
                    # collective; finally cast the summed halves out.
                    for qc in range(S // 128):
                        cs = slice(qc * 128, (qc + 1) * 128)
                        o_tile = o_sb.tile([128, DM], BF16, tag="o_tile")
                        for dmh in range(2):
                            ds_ = slice(dmh * 512, (dmh + 1) * 512)
                            pt = ps.tile([128, 512], F32, tag="ps")
                            for p in range(NP):
                                nc.tensor.matmul(
                                    pt,
                                    lhsT=zT[p][:, cs],
                                    rhs=WOb[p][:, ds_],
                                    start=(p == 0),
                                    stop=(p == NP - 1 and not has_bias),
                                )
                            if has_bias:
                                nc.tensor.matmul(
                                    pt,
                                    lhsT=ones_row[:, :128],
                                    rhs=b_o_half[:, ds_],
                                    start=False,
                                    stop=True,
                                )
                            nc.any.tensor_copy(o_tile[:, ds_], pt)
                        nc.sync.dma_start(rs_in[cs, :], o_tile)
                    if use_cc:
                        nc.gpsimd.collective_compute(
                            "ReduceScatter",
                            mybir.AluOpType.add,
                            replica_groups=GROUPS,
                            ins=[rs_in.opt()],
                            outs=[rs_out.opt()],
                        )
                    else:
                        nc.sync.dma_start(rs_out, rs_in[:SQ, :])
                    for qc in range(S // 128):
                        cs = slice(qc * 128, (qc + 1) * 128)
                        z_sb = o_sb.tile([128, HD_OWN], F32, tag="z_sb")
                        for p in range(NP):
                            ptz = ps_pv.tile([128, 128], BF16, tag="ps_pv")
                            nc.tensor.transpose(ptz, zT[p][:, cs], ident)
                            nc.vector.tensor_copy(
                                z_sb[:, p * 128:(p + 1) * 128], ptz
                            )
                        nc.sync.dma_start(z_d[cs, :], z_sb)
                    for j in range(SQ // 128):
                        r0 = j * 128
                        fb = o_sb.tile([128, DM], BF16, tag="rs_bf")
                        nc.sync.dma_start(fb, rs_out[r0:r0 + 128, :])
                        ff = o_sb.tile([128, DM], F32, tag="rs_f32")
                        nc.vector.tensor_copy(ff, fb)
                        nc.sync.dma_start(out_d[r0:r0 + 128, :], ff)

                for blk in range(2 * NP):
                    p, qh = blk % NP, blk // NP
                    q0 = qh * SQ
                    pv = [[ps_pv.tile([65, 512], F32, tag="ps_pv",
                                      name=f"ps_pv{blk}_{pr}_{qt}")
                           for qt in range(2)] for pr in range(2)]
                    for k in range(KC):
                        ks = slice(k * 128, (k + 1) * 128)
                        for par in range(2):
                            h = 2 * p + par
                            rs = slice(par * 64, (par + 1) * 64)
                            st = ps.tile([128, SQ], F32, tag="ps")
                            for qt in range(2):
                                nc.tensor.matmul(
                                    st[:, qt * 512:(qt + 1) * 512],
                                    lhsT=KTt[p][rs, ks],
                                    rhs=QTt[p][rs, q0 + qt * 512:
                                               q0 + (qt + 1) * 512],
                                    start=True,
                                    stop=True,
                                    tile_position=(par * 64, 0),
                                )
                            pe = p_sb.tile([128, SQ], BF16, tag="p_exp")
                            nc.scalar.activation(
                                pe, st, mybir.ActivationFunctionType.Exp,
                                scale=SCALE,
                            )
                            pm = p_sb.tile([128, SQ], BF16, tag="p_m")
                            nc.vector.tensor_mul(
                                pm, pe, maskTt[k][:, q0:q0 + SQ]
                            )
                            for qt in range(2):
                                nc.tensor.matmul(
                                    pv[par][qt],
                                    lhsT=Vg[k][:, h * 65:(h + 1) * 65],
                                    rhs=pm[:, qt * 512:(qt + 1) * 512],
                                    start=(k == 0),
                                    stop=(k == KC - 1),
                                )
                    for par in range(2):
                        for qt in range(2):
                            qs = slice(q0 + qt * 512, q0 + (qt + 1) * 512)
                            pvc = p_sb.tile([65, 512], F32, tag="pv_sb")
                            nc.scalar.activation(
                                pvc, pv[par][qt],
                                mybir.ActivationFunctionType.Identity,
                            )
                            rr = rrow_pool.tile([1, 512], F32, tag="rrow")
                            nc.vector.reciprocal(rr, pvc[64:65, :])
                            bc = bc_pool.tile([64, 512], F32, tag="bc")
                            nc.gpsimd.partition_broadcast(bc, rr)
                            nc.vector.tensor_mul(
                                zT[p][par * 64:(par + 1) * 64, qs],
                                pvc[0:64, :],
                                bc,
                            )
              if phases >= 3:
                with tc.tile_pool(name="o_sb", bufs=3) as o_sb:
                    phase3_all(o_sb)

    nc.compile()
    return nc


def _make_runner(nc):
    """Persistent jitted SPMD callable mirroring bass2jax.run_bass_via_pjrt."""
    import jax
    from jax.experimental.shard_map import shard_map
    from jax.sharding import Mesh, PartitionSpec, NamedSharding
    from concourse import bass2jax
    from concourse.bass2jax import _bass_exec_p, partition_id_tensor
    import concourse.mybir as mybir_

    bass2jax.install_neuronx_cc_hook()
    n_cores = 8
    partition_name = (
        nc.partition_id_tensor.name if nc.partition_id_tensor else None
    )
    in_names, out_names, out_avals, zero_outs = [], [], [], []
    for alloc in nc.m.functions[0].allocations:
        if not isinstance(alloc, mybir_.MemoryLocationSet):
            continue
        name = alloc.memorylocations[0].name
        if alloc.kind == "ExternalInput":
            if name != partition_name:
                in_names.append(name)
        elif alloc.kind == "ExternalOutput":
            out_names.append(name)
            shape = tuple(alloc.tensor_shape)
            dtype = mybir_.dt.np(alloc.dtype)
            out_avals.append(jax.core.ShapedArray(shape, dtype))
            zero_outs.append(np.zeros(shape, dtype))
    n_params = len(in_names)
    n_outs = len(out_avals)
    all_in_names = list(in_names) + list(out_names)
    if partition_name is not None:
        all_in_names.append(partition_name)
    donate = tuple(range(n_params, n_params + n_outs))

    def _body(*args):
        operands = list(args)
        if partition_name is not None:
            operands.append(partition_id_tensor())
        outs = _bass_exec_p.bind(
            *operands,
            out_avals=tuple(out_avals),
            in_names=tuple(all_in_names),
            out_names=tuple(out_names),
            lowering_input_output_aliases=(),
            sim_require_finite=True,
            sim_require_nnan=True,
            nc=nc,
        )
        return tuple(outs)

    devices = jax.devices()[:n_cores]
    mesh = Mesh(np.asarray(devices), ("core",))
    sharded = jax.jit(
        shard_map(
            _body, mesh=mesh,
            in_specs=(PartitionSpec("core"),) * (n_params + n_outs),
            out_specs=(PartitionSpec("core"),) * n_outs,
            check_rep=False,
        ),
        donate_argnums=donate,
        keep_unused=True,
    )

    def run(in_maps, bench_iters=0):
        import time as _time
        concat_in = [
            np.concatenate([np.asarray(m[name]) for m in in_maps], axis=0)
            for name in in_names
        ]
        concat_zeros = [
            np.zeros((n_cores * z.shape[0], *z.shape[1:]), z.dtype)
            for z in zero_outs
        ]
        sh = NamedSharding(mesh, PartitionSpec("core"))
        ins_dev = [jax.device_put(a, sh) for a in concat_in]
        out_arrs = sharded(*ins_dev, *concat_zeros)
        jax.block_until_ready(out_arrs)
        results = [
            {
                name: np.asarray(out_arrs[i]).reshape(
                    n_cores, *out_avals[i].shape
                )[c]
                for i, name in enumerate(out_names)
            }
            for c in range(n_cores)
        ]
        times = []
        for _ in range(bench_iters):
            zs = [jax.device_put(np.zeros(
                (n_cores * z.shape[0], *z.shape[1:]), z.dtype), sh)
                for z in zero_outs]
            jax.block_until_ready(zs)
            t0 = _time.perf_counter()
            o = sharded(*ins_dev, *zs)
            jax.block_until_ready(o)
            times.append(_time.perf_counter() - t0)
        return results, (min(times) if times else None)

    return run


def kernel(x_q, x_kv, mask, W_Q, W_K, W_V, W_O, b_Q, b_K, b_V, b_O):
    global last_exec_time_ns, last_results
    import os
    x_q = np.ascontiguousarray(np.asarray(x_q, dtype=np.float32))
    x_kv = np.ascontiguousarray(np.asarray(x_kv, dtype=np.float32))
    mask = np.asarray(mask)
    W_Q, W_K, W_V, W_O = [
        np.ascontiguousarray(np.asarray(w, dtype=np.float32))
        for w in (W_Q, W_K, W_V, W_O)
    ]
    bs = [np.ascontiguousarray(np.asarray(b, dtype=np.float32))
          for b in (b_Q, b_K, b_V, b_O)]
    has_bias = bool(any(np.any(b) for b in bs))

    if has_bias not in _cache:
        _cache[has_bias] = _build(has_bias)
    nc = _cache[has_bias]

    # 0/1 keep-mask transposed to [k, q] (identical on every core)
    keepT = np.ascontiguousarray((~mask).T).astype(ml_dtypes.bfloat16)
    in_maps = []
    for c in range(8):
        b_i, hg = divmod(c, 2)
        hs = slice(hg * NHO, (hg + 1) * NHO)
        m = {
            "x_q": x_q[b_i],
            "x_kv": x_kv[b_i],
            "maskT": keepT,
            "W_Q": np.ascontiguousarray(W_Q[hs]),
            "W_K": np.ascontiguousarray(W_K[hs]),
            "W_V": np.ascontiguousarray(W_V[hs]),
            "W_O": np.ascontiguousarray(W_O[hs]),
        }
        if has_bias:
            m.update({
                "b_Q": np.ascontiguousarray(bs[0][hs]),
                "b_K": np.ascontiguousarray(bs[1][hs]),
                "b_V": np.ascontiguousarray(bs[2][hs]),
                "b_O": bs[3],
            })
        in_maps.append(m)

    key = ("runner", has_bias)
    if key not in _cache:
        _cache[key] = _make_runner(nc)
    bench_iters = int(os.environ.get("BENCH_ITERS", "0"))
    results, tmin = _cache[key](in_maps, bench_iters=bench_iters)
    last_exec_time_ns = None if tmin is None else int(tmin * 1e9)
    last_results = results

    out = np.empty((B, S, DM), np.float32)
    z = np.empty((B, S, NH, DH), np.float32)
    for c in range(8):
        b_i, hg = divmod(c, 2)
        out[b_i, hg * SQ:(hg + 1) * SQ, :] = results[c]["out"]
        z[b_i, :, hg * NHO:(hg + 1) * NHO, :] = (
            results[c]["z"].reshape(S, NHO, DH)
        )
    return out, z
